# revision 64
# baseline (speedup 1.0000x reference)
"""DGCNN (nn_DGCNN_type1) Trainium2 Bass kernel — self-contained.

Strategy: data-parallel over the 128 graphs, 16 per NeuronCore across 8 cores.
Per graph the chain is score-matmul -> DVE top-16 -> dma_gather -> edge MLP
(PSUM-accumulated f16 matmuls) -> k-max -> next conv -> lin1 + pool.  The
graph loop is software-pipelined two graphs deep at emission order so the
11.7us gathers and the DVE topk chain of graph t overlap the PE/Act edge-MLP
work of graphs t-1 / t-2.  All score math is f16 (PSUM f32 accumulate);
PSUM-releasing DVE ops (k-max, pool reduce) are emitted ahead of the topk
chains so PSUM pool rotation never waits on a topk.
"""

import numpy as np
import concourse.bacc as bacc
import concourse.mybir as mybir
from concourse.tile import TileContext
from concourse.masks import make_identity

F32, F16, I16, U16 = (mybir.dt.float32, mybir.dt.float16, mybir.dt.int16,
                      mybir.dt.uint16)
AF = mybir.ActivationFunctionType
ALU = mybir.AluOpType
AX = mybir.AxisListType

N = 512
K = 16
NCHUNK = N // 128  # 4 row-chunks for the NxN score matrix


def host_prep(inputs, G, core):
    """Build the per-core in_map (numpy only: layout/dtype prep, no model math)."""
    f16 = np.float16
    x, pos, tq = inputs["x"], inputs["pos"], inputs["tq"]
    B_all = x.shape[0] // N
    xx = np.concatenate([tq, x, pos], axis=1).reshape(B_all, N, 5).astype(np.float32)
    sl = slice(core * G, (core + 1) * G)
    xxc = xx[sl]                                   # [G, 512, 5]
    feat5 = np.ascontiguousarray(xxc.transpose(0, 2, 1))         # [G, 5, 512]
    xtab1 = np.zeros((G, N, 128), f16)
    xtab1[:, :, 0:5] = xxc.astype(f16)

    w1a, w1b = inputs["w1a"], inputs["w1b"]
    w2a, w2b = inputs["w2a"], inputs["w2b"]
    w1botp = np.zeros((128, 64), f16)
    w1botp[0:5] = w1a[5:10].astype(f16)
    w2botp = np.zeros((128, 128), f16)
    w2botp[0:64] = w2a[64:128].astype(f16)
    wl1 = inputs["wl1"]

    return {
        "feat5h": feat5.astype(f16),
        "xtab1": xtab1,
        "w1modz": np.ascontiguousarray(
            np.vstack([np.zeros((1, 64), np.float32),
                       w1a[0:5] - w1a[5:10]]).astype(f16)),
        "w1botp": w1botp,
        "w1b": np.ascontiguousarray(w1b.astype(f16)),
        "b1a2": np.tile(inputs["b1a"], 2).reshape(128, 1).astype(np.float32),
        "w1b2": np.ascontiguousarray(np.vstack([w1b, w1b]).astype(f16)),
        "b1b": inputs["b1b"].reshape(64, 1).astype(np.float32),
        "w2mod": np.ascontiguousarray((w2a[0:64] - w2a[64:128]).astype(f16)),
        "w2botp": w2botp,
        "w2b": np.ascontiguousarray(w2b.astype(f16)),
        "b2a": inputs["b2a"].reshape(128, 1).astype(np.float32),
        "b2b": inputs["b2b"].reshape(64, 1).astype(np.float32),
        "wl1xxz": np.ascontiguousarray(
            np.vstack([np.zeros((1, 512), np.float32), wl1[0:5]]).astype(f16)),
        "wl1x1": np.ascontiguousarray(wl1[5:69].astype(f16)),
        "wl1x2": np.ascontiguousarray(wl1[69:133].astype(f16)),
        "bl1c": np.ascontiguousarray(inputs["bl1"].reshape(4, 128).T.astype(np.float32)),
        "wl2": np.ascontiguousarray(inputs["wl2"].astype(f16).reshape(4, 128, 256).transpose(1, 0, 2)),
        "bl2c": np.ascontiguousarray(inputs["bl2"].reshape(2, 128).T.astype(np.float32)),
        "wm1": np.ascontiguousarray(inputs["wm1"].astype(f16).reshape(2, 128, 128).transpose(1, 0, 2)),
        "bm1": inputs["bm1"].reshape(128, 1).astype(np.float32),
        "wm2": np.ascontiguousarray(inputs["wm2"].astype(f16)),
        "bm2": inputs["bm2"].reshape(3, 1).astype(np.float32),
    }


def declare_io(nc, G):
    t = {}
    def inp(name, shape, dt):
        t[name] = nc.dram_tensor(name, shape, dt, kind="ExternalInput")
    inp("feat5h", [G, 5, N], F16)
    inp("xtab1", [G, N, 128], F16)
    inp("w1modz", [6, 64], F16); inp("w1botp", [128, 64], F16)
    inp("w1b", [64, 64], F16); inp("w1b2", [128, 64], F16)
    inp("b1a2", [128, 1], F32); inp("b1b", [64, 1], F32)
    inp("w2mod", [64, 128], F16); inp("w2botp", [128, 128], F16)
    inp("w2b", [128, 64], F16); inp("b2a", [128, 1], F32); inp("b2b", [64, 1], F32)
    inp("wl1xxz", [6, N], F16); inp("wl1x1", [64, N], F16); inp("wl1x2", [64, N], F16)
    inp("bl1c", [128, 4], F32); inp("wl2", [128, 4, 256], F16); inp("bl2c", [128, 2], F32)
    inp("wm1", [128, 2, 128], F16); inp("bm1", [128, 1], F32)
    inp("wm2", [128, 3], F16); inp("bm2", [3, 1], F32)
    t["o"] = nc.dram_tensor("o", [3, G], F32, kind="ExternalOutput")
    return t


def build(nc, G, reps=1):
    t = declare_io(nc, G)
    with TileContext(nc) as tc:
        _build_body(nc, tc, t, G, reps)
    nc.compile()
    return t


def _build_body(nc, tc, t, G, reps=1):
    sbw = tc.alloc_tile_pool(name="sbw", bufs=1)          # persistent
    sb = tc.alloc_tile_pool(name="sb", bufs=2)            # rotating tiles
    ps = tc.alloc_tile_pool(name="ps", bufs=1, space="PSUM")
    dram = tc.alloc_tile_pool(name="dram", bufs=1, space="DRAM")

    # ---- persistent weight tiles ----
    w = {}
    for name in ["w1modz", "w1botp", "w1b", "w1b2", "w2mod", "w2botp", "w2b",
                 "wl1xxz", "wl1x1", "wl1x2", "wl2", "wm1", "wm2"]:
        w[name] = sbw.tile(list(t[name].shape), F16, tag=name, name='w_' + name)
        nc.sync.dma_start(out=w[name][:], in_=t[name][:])
    for name in ["b1a2", "b1b", "b2a", "b2b", "bl1c", "bl2c", "bm1", "bm2"]:
        w[name] = sbw.tile(list(t[name].shape), F32, tag=name, name='b_' + name)
        nc.sync.dma_start(out=w[name][:], in_=t[name][:])
    ident = sbw.tile([128, 128], F16, tag="ident")
    make_identity(nc, ident[:])
    ones64 = sbw.tile([64, 1], F16, tag="ones64")
    nc.gpsimd.memset(ones64[:], 1.0)
    ones6z = sbw.tile([6, 1], F16, tag="ones6z")
    nc.gpsimd.memset(ones6z[:], 1.0)
    nc.gpsimd.memset(ones6z[0:1, :], 0.0)
    onesrow = sbw.tile([1, N], F16, tag="onesrow")
    nc.gpsimd.memset(onesrow[:], 1.0)

    # persistent gather-index tiles (rows 32-127 must hold valid values)
    NIDX_SLOTS = 3
    idx_tiles = []
    for s in range(NIDX_SLOTS):
        it = sbw.tile([128, N], I16, tag=f"idxs{s}", name=f"idxs{s}")
        nc.gpsimd.memset(it[:], 0)
        idx_tiles.append(it)
    idx_slot = [0]

    x1tab = dram.tile([G, N, 128], F16, tag="x1tab")

    Gt_lo = sbw.tile([128, G], F32, tag="gtlo")
    Gt_hi = sbw.tile([128, G], F32, tag="gthi")

    rep_ctx = tc.For_i(0, reps, 1) if reps > 1 else None
    if rep_ctx is not None:
        rep_ctx.__enter__()

    st = {}  # per-graph in-flight tiles

    # ----------------- stage bodies -----------------
    def score_block(g, dp, lhs_sc, rhs_B, nd_tag):
        """Emit sq row + 4 chunk score matmuls; returns list of SBUF nd tiles.
        score[p, f] = feat_p . feat_f - |feat_f|^2 / 2  (ordering == -d2/2)."""
        nds = []
        for c in range(NCHUNK):
            nd_p = ps.tile([128, 1024], F32, tag="a1", bufs=2, name="ndp")
            nc.tensor.matmul(out=nd_p[:, 0:N],
                             lhsT=lhs_sc[0:dp + 1, 128 * c:128 * (c + 1)],
                             rhs=rhs_B[0:dp + 1, :], start=True, stop=True)
            nd = sb.tile([128, N], F32, tag=nd_tag,
                         bufs=(9 if nd_tag == "nd1" else 5), name=nd_tag)
            nc.scalar.activation(nd[:], nd_p[:, 0:N], AF.Copy)
            nds.append(nd)
        return nds

    class _TopkChunk:
        """One 128-node topk chunk.  .dve() emits the DVE top-16 ops; .chain()
        emits idx transpose -> idx copy -> quarter dma_gather.  The two parts
        are emitted at different points of the consumer W-stage so the PE
        transpose never head-of-line-blocks W matmuls (and vice versa)."""

        def __init__(self, nd, idxTp, idxs, gtab_ap, xjg, c, sbuf_src=False):
            self.__dict__.update(nd=nd, idxTp=idxTp, idxs=idxs,
                                 gtab_ap=gtab_ap, xjg=xjg, c=c,
                                 sbuf_src=sbuf_src)

        def dve(self):
            nd = self.nd
            maxv = sb.tile([128, 16], F32, tag="maxv", name="maxv")
            maxi = sb.tile([128, 16], U16, tag="maxi", name="maxi")
            nc.vector.max(out=maxv[:, 0:8], in_=nd[:])
            nc.vector.max_index(out=maxi[:, 0:8], in_max=maxv[:, 0:8], in_values=nd[:])
            nc.vector.match_replace(out=nd[:], in_to_replace=maxv[:, 0:8],
                                    in_values=nd[:], imm_value=-1e30)
            nc.vector.max(out=maxv[:, 8:16], in_=nd[:])
            nc.vector.max_index(out=maxi[:, 8:16], in_max=maxv[:, 8:16], in_values=nd[:])
            mif = sb.tile([128, 16], F16, tag="mif", name="mif")
            nc.vector.tensor_copy(mif[:], maxi[:])
            self.mif = mif

        def chain(self):
            """Transpose this chunk's indices; on odd chunks launch a
            half-gather covering chunks c-1 and c (fewer cross-engine hops
            than per-chunk gathers — HW latency is hop-dominated)."""
            QI = K * 128
            c, idxs = self.c, self.idxs
            cs = slice(128 * c, 128 * (c + 1))
            nc.tensor.transpose(out=self.idxTp[:, cs], in_=self.mif[:],
                                identity=ident[:])
            if c % 2 == 1:
                hs = slice(128 * (c - 1), 128 * (c + 1))
                nc.scalar.activation(idxs[0:16, hs], self.idxTp[:, hs], AF.Copy)
                nc.sync.dma_start(out=idxs[16:32, hs], in_=idxs[0:16, hs])
                kw = {}
                if self.sbuf_src:
                    kw = dict(sbuf_tokens_per_rank=128,
                              sbuf_free_dim_per_rank=256,
                              sbuf_free_dim_pad_per_rank=0,
                              sbuf_byte_offset=0)
                nc.gpsimd.dma_gather(
                    out_ap=self.xjg[:, None, QI * (c - 1):QI * (c + 1)],
                    in_ap=self.gtab_ap, idxs_ap=idxs[:, hs],
                    num_idxs=2 * QI, num_idxs_reg=2 * QI, elem_size=128,
                    transpose=True, single_packet=False, **kw)

    def make_topk_chunks(nds, gtab_ap, xjg_tag, sbuf_src=False):
        xjg = sb.tile([128, K * N], F16, tag=xjg_tag, name=xjg_tag)
        idxTp = ps.tile([16, N], F16, tag="idxTp", name="idxTp")
        idxs = idx_tiles[idx_slot[0] % NIDX_SLOTS]
        idx_slot[0] += 1
        return xjg, [_TopkChunk(nds[c], idxTp, idxs, gtab_ap, xjg, c, sbuf_src)
                     for c in range(NCHUNK)]

    # ----------------- A1: conv1 score prep -----------------
    # B1 layout: row 0 = -|f|^2/2 (Act-writable partition 0), rows 1:6 = feat.
    # Zero rows in w1modz / wl1xxz / ones6z null out row 0 where unwanted.
    def prefetch_B(g):
        B = sb.tile([6, N], F16, tag="B1", bufs=5, name="B1")
        nc.sync.dma_start(out=B[1:6, :], in_=t["feat5h"][g])
        nc.gpsimd.memset(B[0:1, :], 0.0)
        st.setdefault(g, {})["B1"] = B

    def A1a(g):
        if g + 1 < G:
            prefetch_B(g + 1)
        B = st[g]["B1"]
        sc = sb.tile([6, N], F16, tag="sc1", name="sc1")
        nc.scalar.activation(sc[:], B[:], AF.Copy)
        nc.scalar.activation(sc[0:1, :], onesrow[:], AF.Copy)
        F2 = sb.tile([6, N], F16, tag="F21", name="F21")
        nc.scalar.activation(F2[:], B[:], AF.Square)
        sqp = ps.tile([1, N], F32, tag="ms", name="sqp1")
        nc.tensor.matmul(out=sqp[:], lhsT=ones6z[:], rhs=F2[:],
                         start=True, stop=True)
        nc.scalar.activation(B[0:1, :], sqp[:], AF.Copy, scale=-0.5)
        st[g]["nds1"] = score_block(g, 5, sc, B, "nd1")

    # ----------------- W1: conv1 edge MLP -> S2 + x1tab -----------------
    def W1(g):
        B = st[g]["B1"]
        xjg = st[g].pop("xjg1")
        S2 = sb.tile([65, N], F16, tag="S2", bufs=3, name="S2")
        st[g]["S2"] = S2

        a1s, a2s = {}, {}

        def l1(s):
            a1 = ps.tile([128, 1024], F32, tag="a1", bufs=2, name="a1")
            a1s[s] = a1
            for h in range(2):
                for r in range(2):
                    c = 4 * s + 2 * h + r
                    out = a1[64 * h:64 * h + 64, 512 * r:512 * (r + 1)]
                    nc.tensor.matmul(out=out, lhsT=w["w1modz"][:],
                                     rhs=B[0:6, 32 * c:32 * (c + 1), None]
                                     .to_broadcast([6, 32, K]),
                                     start=True, stop=False)
                    nc.tensor.matmul(out=out, lhsT=w["w1botp"][:],
                                     rhs=xjg[:, 512 * c:512 * (c + 1)],
                                     start=False, stop=True)

        def l2(s):
            a1 = a1s.pop(s)
            h1 = sb.tile([128, 1024], F16, tag="h1", bufs=3, name="h1")
            nc.scalar.activation(h1[:], a1[:], AF.Prelu, bias=w["b1a2"][:],
                                 alpha=0.01)
            a2 = ps.tile([128, 1024], F32, tag="a2", name="a2")
            a2s[s] = a2
            for h in range(2):
                for r in range(2):
                    nc.tensor.matmul(
                        out=a2[64 * h:64 * h + 64, 512 * r:512 * (r + 1)],
                        lhsT=w["w1b2"][64 * h:64 * h + 64, :],
                        rhs=h1[64 * h:64 * h + 64, 512 * r:512 * (r + 1)],
                        start=True, stop=True)

        # SBUF-resident conv2 gather table: [p, c, :] = node 128c+p (matches
        # the dma_gather sbuf stripe map with tokens_per_rank=128,
        # free_dim_per_rank=256B) — no DRAM round-trip for x1 at all.
        x1sb = sb.tile([128, 4, 128], F16, tag="x1sb", bufs=2, name="x1sb")
        st[g]["x1sb"] = x1sb

        def kout(tt):
            a2 = a2s.pop(tt)
            kmx = sb.tile([128, 64], F32, tag="kmx", name="kmx")
            nc.vector.tensor_reduce(out=kmx[:], in_=a2[:].rearrange(
                "p (m k) -> p m k", k=K), op=ALU.max, axis=AX.X)
            for h in range(2):
                cols = slice(128 * tt + 64 * h, 128 * tt + 64 * h + 64)
                nc.scalar.activation(S2[0:64, cols], kmx[64 * h:64 * h + 64, :],
                                     AF.Prelu, bias=w["b1b"][:], alpha=0.01)
            # node-major staging straight into the SBUF table
            Tp = ps.tile([128, 64], F16, tag="ms", name="Tp")
            nc.tensor.transpose(out=Tp[:], in_=S2[0:64, 128 * tt:128 * (tt + 1)],
                                identity=ident[0:64, 0:64])
            nc.scalar.activation(x1sb[:, tt, 0:64], Tp[:], AF.Copy)
            nc.scalar.activation(x1sb[:, tt, 64:128], Tp[:], AF.Copy)

        # one-super lookahead so PE never waits on the Act Prelu
        l1(0)
        for s in range(1, 4):
            l1(s)
            l2(s - 1)
            kout(s - 1)
        l2(3)
        kout(3)

    # ----------------- A2: conv2 score / topk+gather -----------------
    def A2a(g):
        S2 = st[g]["S2"]
        sc2 = sb.tile([65, N], F16, tag="sc2", name="sc2")
        nc.scalar.activation(sc2[0:64, :], S2[0:64, :], AF.Copy)
        nc.scalar.activation(sc2[64:65, :], onesrow[:], AF.Copy)
        F22 = sb.tile([64, N], F16, tag="F22", name="F22")
        nc.scalar.activation(F22[:], S2[0:64, :], AF.Square)
        sqp = ps.tile([1, N], F32, tag="ms", name="sqp2")
        nc.tensor.matmul(out=sqp[:], lhsT=ones64[:], rhs=F22[:],
                         start=True, stop=True)
        nc.scalar.activation(S2[64:65, :], sqp[:], AF.Copy, scale=-0.5)
        st[g]["nds2"] = score_block(g, 64, sc2, S2, "nd2")

    # ----------------- W2: conv2 edge MLP -----------------
    def W2a(g):
        S2 = st[g]["S2"]
        xjg = st[g].pop("xjg2")
        x2 = sb.tile([64, N], F16, tag="x2", name="x2")
        st[g]["x2"] = x2

        a1s, a2s = {}, {}

        def l1(s):
            a1 = ps.tile([128, 1024], F32, tag="a1", bufs=2, name="a1c2")
            a1s[s] = a1
            for r in range(2):
                c = 2 * s + r
                out = a1[:, 512 * r:512 * (r + 1)]
                nc.tensor.matmul(out=out, lhsT=w["w2mod"][:],
                                 rhs=S2[0:64, 32 * c:32 * (c + 1), None]
                                 .to_broadcast([64, 32, K]),
                                 start=True, stop=False)
                nc.tensor.matmul(out=out, lhsT=w["w2botp"][:],
                                 rhs=xjg[:, 512 * c:512 * (c + 1)],
                                 start=False, stop=True)

        def l2(s):
            a1 = a1s.pop(s)
            h1 = sb.tile([128, 1024], F16, tag="h1", bufs=3, name="h1c2")
            nc.scalar.activation(h1[:], a1[:], AF.Prelu, bias=w["b2a"][:],
                                 alpha=0.01)
            if s % 2 == 0:
                a2s[s // 2] = ps.tile([128, 1024], F32, tag="a2", name="a2c2")
            a2 = a2s[s // 2]
            hh = s % 2
            for r in range(2):
                nc.tensor.matmul(
                    out=a2[64 * hh:64 * hh + 64, 512 * r:512 * (r + 1)],
                    lhsT=w["w2b"][:], rhs=h1[:, 512 * r:512 * (r + 1)],
                    start=True, stop=True)

        def kout(tt):
            a2 = a2s.pop(tt)
            kmx = sb.tile([128, 64], F32, tag="kmx", name="kmx2")
            nc.vector.tensor_reduce(out=kmx[:], in_=a2[:].rearrange(
                "p (m k) -> p m k", k=K), op=ALU.max, axis=AX.X)
            for h in range(2):
                cols = slice(128 * tt + 64 * h, 128 * tt + 64 * h + 64)
                nc.scalar.activation(x2[:, cols], kmx[64 * h:64 * h + 64, :],
                                     AF.Prelu, bias=w["b2b"][:], alpha=0.01)

        l1(0)
        for s in range(1, 8):
            l1(s)
            l2(s - 1)
            if (s - 1) % 2 == 1:
                kout((s - 1) // 2)
        l2(7)
        kout(3)

    # ----------------- lin1 + global max pool -----------------
    def LIN(g):
        B = st[g]["B1"]
        S2 = st[g]["S2"]
        x2 = st[g]["x2"]
        hsbs = []
        for c in range(NCHUNK):
            hp = ps.tile([128, N], F32, tag="a1", bufs=2, name="hp")
            nc.tensor.matmul(out=hp[:], lhsT=w["wl1xxz"][:, 128 * c:128 * (c + 1)],
                             rhs=B[0:6, :], start=True, stop=False)
            nc.tensor.matmul(out=hp[:], lhsT=w["wl1x1"][:, 128 * c:128 * (c + 1)],
                             rhs=S2[0:64, :], start=False, stop=False)
            nc.tensor.matmul(out=hp[:], lhsT=w["wl1x2"][:, 128 * c:128 * (c + 1)],
                             rhs=x2[:], start=False, stop=True)
            hsb = sb.tile([128, N], F16, tag="hsb", bufs=8, name="hsb")
            nc.scalar.activation(hsb[:], hp[:], AF.Prelu,
                                 bias=w["bl1c"][:, c:c + 1], alpha=0.01)
            hsbs.append(hsb)
        for fo in range(2):
            h2p = ps.tile([128, N], F32, tag="a2", name="h2p")
            for c in range(NCHUNK):
                nc.tensor.matmul(out=h2p[:],
                                 lhsT=w["wl2"][:, c, 128 * fo:128 * (fo + 1)],
                                 rhs=hsbs[c][:], start=(c == 0),
                                 stop=(c == NCHUNK - 1))
            gt = Gt_lo if fo == 0 else Gt_hi
            nc.vector.tensor_reduce(out=gt[:, g:g + 1], in_=h2p[:], op=ALU.max,
                                    axis=AX.X)
        del st[g]

    # ----------------- software-pipelined graph loop -----------------
    # A1a(g) runs a full tstep before graph g's topk chunks so the score-prep
    # chain (Pool copies -> sqp -> B0 -> score mms -> nd copies) is never on
    # the iteration-boundary critical path.
    tk1, tk2 = {}, {}
    prefetch_B(0)
    A1a(0)
    for tstep in range(G + 2):
        if tstep + 1 < G:
            A1a(tstep + 1)
        if tstep < G:
            xjg, cks = make_topk_chunks(st[tstep].pop("nds1"),
                                        t["xtab1"][tstep], "xjg1")
            st[tstep]["xjg1"] = xjg
            tk1[tstep] = cks
        if 0 <= tstep - 1 < G:
            W1(tstep - 1)
            A2a(tstep - 1)
            xjg, cks2 = make_topk_chunks(st[tstep - 1].pop("nds2"),
                                         st[tstep - 1]["x1sb"][:], "xjg2",
                                         sbuf_src=True)
            st[tstep - 1]["xjg2"] = xjg
            tk2[tstep - 1] = cks2
        if 0 <= tstep - 2 < G:
            W2a(tstep - 2)
        for ck in tk1.pop(tstep, []):
            ck.dve()
            ck.chain()
        for ck in tk2.pop(tstep - 1, []):
            ck.dve()
            ck.chain()
        if 0 <= tstep - 2 < G:
            LIN(tstep - 2)

    # ----------------- head -----------------
    t1p = ps.tile([128, G], F32, tag="ms", name="t1p")
    for fo in range(2):
        gt = Gt_lo if fo == 0 else Gt_hi
        ga = sb.tile([128, G], F16, tag="ga", name="ga")
        nc.scalar.activation(ga[:], gt[:], AF.Prelu, bias=w["bl2c"][:, fo:fo + 1],
                             alpha=0.01)
        nc.tensor.matmul(out=t1p[:], lhsT=w["wm1"][:, fo, :],
                         rhs=ga[:], start=(fo == 0), stop=(fo == 1))
    t1 = sb.tile([128, G], F16, tag="t1", name="t1")
    nc.scalar.activation(t1[:], t1p[:], AF.Prelu, bias=w["bm1"][:], alpha=0.01)
    outp = ps.tile([3, G], F32, tag="ms", name="outp")
    nc.tensor.matmul(out=outp[:], lhsT=w["wm2"][:], rhs=t1[:], start=True, stop=True)
    outsb = sb.tile([3, G], F32, tag="outsb", name="outsb")
    nc.scalar.activation(outsb[:], outp[:], AF.Identity, bias=w["bm2"][:])
    nc.sync.dma_start(out=t["o"][:], in_=outsb[:])

    if rep_ctx is not None:
        rep_ctx.__exit__(None, None, None)

    for pool in (dram, ps, sb, sbw):
        pool.release()


# ======================= harness entry point =======================
_CACHE = {}


def _get_program(G):
    if "nc" not in _CACHE:
        import concourse.bacc as _bacc
        nc = _bacc.Bacc()
        build(nc, G)
        _CACHE["nc"] = nc
    return _CACHE["nc"]


def kernel(x, pos, tq, batch, w1a, b1a, w1b, b1b, w2a, b2a, w2b, b2b,
           wl1, bl1, wl2, bl2, wm1, bm1, wm2, bm2):
    """Full-input entry: shards graphs over 8 NeuronCores, returns [128, 3]."""
    from concourse.bass_utils import run_bass_kernel_spmd
    inputs = dict(x=np.asarray(x), pos=np.asarray(pos), tq=np.asarray(tq),
                  w1a=np.asarray(w1a), b1a=np.asarray(b1a),
                  w1b=np.asarray(w1b), b1b=np.asarray(b1b),
                  w2a=np.asarray(w2a), b2a=np.asarray(b2a),
                  w2b=np.asarray(w2b), b2b=np.asarray(b2b),
                  wl1=np.asarray(wl1), bl1=np.asarray(bl1),
                  wl2=np.asarray(wl2), bl2=np.asarray(bl2),
                  wm1=np.asarray(wm1), bm1=np.asarray(bm1),
                  wm2=np.asarray(wm2), bm2=np.asarray(bm2))
    NCORES = 8
    B_all = inputs["x"].shape[0] // N
    G = B_all // NCORES
    nc = _get_program(G)
    in_maps = [host_prep(inputs, G, c) for c in range(NCORES)]
    res = run_bass_kernel_spmd(nc, in_maps, core_ids=list(range(NCORES)))
    out = np.concatenate([res.results[c]["o"].T for c in range(NCORES)], axis=0)
    return out.astype(np.float32)


# revision 66
# speedup vs baseline: 1.1474x; 1.1474x over previous
"""DGCNN (nn_DGCNN_type1) Trainium2 Bass kernel — self-contained.

Strategy: data-parallel over the 128 graphs, 16 per NeuronCore across 8 cores.
Per graph the chain is score-matmul -> DVE top-16 -> dma_gather -> edge MLP
(PSUM-accumulated f16 matmuls) -> k-max -> next conv -> lin1 + pool.  The
graph loop is software-pipelined two graphs deep at emission order so the
11.7us gathers and the DVE topk chain of graph t overlap the PE/Act edge-MLP
work of graphs t-1 / t-2.  All score math is f16 (PSUM f32 accumulate);
PSUM-releasing DVE ops (k-max, pool reduce) are emitted ahead of the topk
chains so PSUM pool rotation never waits on a topk.
"""

import numpy as np
import concourse.bacc as bacc
import concourse.mybir as mybir
from concourse.tile import TileContext
from concourse.masks import make_identity

F32, F16, I16, U16 = (mybir.dt.float32, mybir.dt.float16, mybir.dt.int16,
                      mybir.dt.uint16)
AF = mybir.ActivationFunctionType
ALU = mybir.AluOpType
AX = mybir.AxisListType

N = 512
K = 16
NCHUNK = N // 128  # 4 row-chunks for the NxN score matrix


def host_prep(inputs, G, core):
    """Build the per-core in_map (numpy only: layout/dtype prep, no model math)."""
    f16 = np.float16
    x, pos, tq = inputs["x"], inputs["pos"], inputs["tq"]
    B_all = x.shape[0] // N
    xx = np.concatenate([tq, x, pos], axis=1).reshape(B_all, N, 5).astype(np.float32)
    sl = slice(core * G, (core + 1) * G)
    xxc = xx[sl]                                   # [G, 512, 5]
    feat5 = np.ascontiguousarray(xxc.transpose(0, 2, 1))         # [G, 5, 512]
    xtab1 = np.zeros((G, N, 128), f16)
    xtab1[:, :, 0:5] = xxc.astype(f16)

    w1a, w1b = inputs["w1a"], inputs["w1b"]
    w2a, w2b = inputs["w2a"], inputs["w2b"]
    w1botp = np.zeros((128, 64), f16)
    w1botp[0:5] = w1a[5:10].astype(f16)
    w2botp = np.zeros((128, 128), f16)
    w2botp[0:64] = w2a[64:128].astype(f16)
    wl1 = inputs["wl1"]

    return {
        "feat5h": feat5.astype(f16),
        "xtab1": xtab1,
        "w1modz": np.ascontiguousarray(
            np.vstack([np.zeros((1, 64), np.float32),
                       w1a[0:5] - w1a[5:10]]).astype(f16)),
        "w1botp": w1botp,
        "w1b": np.ascontiguousarray(w1b.astype(f16)),
        "b1a2": np.tile(inputs["b1a"], 2).reshape(128, 1).astype(np.float32),
        "w1b2": np.ascontiguousarray(np.vstack([w1b, w1b]).astype(f16)),
        "b1b": inputs["b1b"].reshape(64, 1).astype(np.float32),
        "w2mod": np.ascontiguousarray((w2a[0:64] - w2a[64:128]).astype(f16)),
        "w2botp": w2botp,
        "w2b": np.ascontiguousarray(w2b.astype(f16)),
        "b2a": inputs["b2a"].reshape(128, 1).astype(np.float32),
        "b2b": inputs["b2b"].reshape(64, 1).astype(np.float32),
        "wl1xxz": np.ascontiguousarray(
            np.vstack([np.zeros((1, 512), np.float32), wl1[0:5]]).astype(f16)),
        "wl1x1": np.ascontiguousarray(wl1[5:69].astype(f16)),
        "wl1x2": np.ascontiguousarray(wl1[69:133].astype(f16)),
        "bl1c": np.ascontiguousarray(inputs["bl1"].reshape(4, 128).T.astype(np.float32)),
        "wl2": np.ascontiguousarray(inputs["wl2"].astype(f16).reshape(4, 128, 256).transpose(1, 0, 2)),
        "bl2c": np.ascontiguousarray(inputs["bl2"].reshape(2, 128).T.astype(np.float32)),
        "wm1": np.ascontiguousarray(inputs["wm1"].astype(f16).reshape(2, 128, 128).transpose(1, 0, 2)),
        "bm1": inputs["bm1"].reshape(128, 1).astype(np.float32),
        "wm2": np.ascontiguousarray(inputs["wm2"].astype(f16)),
        "bm2": inputs["bm2"].reshape(3, 1).astype(np.float32),
    }


def declare_io(nc, G):
    t = {}
    def inp(name, shape, dt):
        t[name] = nc.dram_tensor(name, shape, dt, kind="ExternalInput")
    inp("feat5h", [G, 5, N], F16)
    inp("xtab1", [G, N, 128], F16)
    inp("w1modz", [6, 64], F16); inp("w1botp", [128, 64], F16)
    inp("w1b", [64, 64], F16); inp("w1b2", [128, 64], F16)
    inp("b1a2", [128, 1], F32); inp("b1b", [64, 1], F32)
    inp("w2mod", [64, 128], F16); inp("w2botp", [128, 128], F16)
    inp("w2b", [128, 64], F16); inp("b2a", [128, 1], F32); inp("b2b", [64, 1], F32)
    inp("wl1xxz", [6, N], F16); inp("wl1x1", [64, N], F16); inp("wl1x2", [64, N], F16)
    inp("bl1c", [128, 4], F32); inp("wl2", [128, 4, 256], F16); inp("bl2c", [128, 2], F32)
    inp("wm1", [128, 2, 128], F16); inp("bm1", [128, 1], F32)
    inp("wm2", [128, 3], F16); inp("bm2", [3, 1], F32)
    t["o"] = nc.dram_tensor("o", [3, G], F32, kind="ExternalOutput")
    return t


def build(nc, G, reps=1):
    t = declare_io(nc, G)
    with TileContext(nc) as tc:
        _build_body(nc, tc, t, G, reps)
    nc.compile()
    return t


def _build_body(nc, tc, t, G, reps=1):
    sbw = tc.alloc_tile_pool(name="sbw", bufs=1)          # persistent
    sb = tc.alloc_tile_pool(name="sb", bufs=2)            # rotating tiles
    ps = tc.alloc_tile_pool(name="ps", bufs=1, space="PSUM")
    dram = tc.alloc_tile_pool(name="dram", bufs=1, space="DRAM")

    # ---- persistent weight tiles ----
    w = {}
    for name in ["w1modz", "w1botp", "w1b", "w1b2", "w2mod", "w2botp", "w2b",
                 "wl1xxz", "wl1x1", "wl1x2", "wl2", "wm1", "wm2"]:
        w[name] = sbw.tile(list(t[name].shape), F16, tag=name, name='w_' + name)
        nc.sync.dma_start(out=w[name][:], in_=t[name][:])
    for name in ["b1a2", "b1b", "b2a", "b2b", "bl1c", "bl2c", "bm1", "bm2"]:
        w[name] = sbw.tile(list(t[name].shape), F32, tag=name, name='b_' + name)
        nc.sync.dma_start(out=w[name][:], in_=t[name][:])
    ident = sbw.tile([128, 128], F16, tag="ident")
    make_identity(nc, ident[:])
    ones64 = sbw.tile([64, 1], F16, tag="ones64")
    nc.gpsimd.memset(ones64[:], 1.0)
    ones6z = sbw.tile([6, 1], F16, tag="ones6z")
    nc.gpsimd.memset(ones6z[:], 1.0)
    nc.gpsimd.memset(ones6z[0:1, :], 0.0)
    onesrow = sbw.tile([1, N], F16, tag="onesrow")
    nc.gpsimd.memset(onesrow[:], 1.0)

    # persistent gather-index tiles (rows 32-127 must hold valid values)
    NIDX_SLOTS = 4
    idx_tiles = []
    for s in range(NIDX_SLOTS):
        it = sbw.tile([128, N], I16, tag=f"idxs{s}", name=f"idxs{s}")
        nc.gpsimd.memset(it[:], 0)
        idx_tiles.append(it)
    idx_slot = [0]

    x1tab = dram.tile([G, N, 128], F16, tag="x1tab")

    Gt_lo = sbw.tile([128, G], F32, tag="gtlo")
    Gt_hi = sbw.tile([128, G], F32, tag="gthi")

    rep_ctx = tc.For_i(0, reps, 1) if reps > 1 else None
    if rep_ctx is not None:
        rep_ctx.__enter__()

    st = {}  # per-graph in-flight tiles

    # ----------------- stage bodies -----------------
    def score_block(g, dp, lhs_sc, rhs_B, nd_tag):
        """Emit sq row + 4 chunk score matmuls; returns list of SBUF nd tiles.
        score[p, f] = feat_p . feat_f - |feat_f|^2 / 2  (ordering == -d2/2)."""
        nds = []
        for c in range(NCHUNK):
            nd_p = ps.tile([128, 1024], F32, tag="a1", bufs=2, name="ndp")
            nc.tensor.matmul(out=nd_p[:, 0:N],
                             lhsT=lhs_sc[0:dp + 1, 128 * c:128 * (c + 1)],
                             rhs=rhs_B[0:dp + 1, :], start=True, stop=True)
            nd = sb.tile([128, N], F32, tag=nd_tag,
                         bufs=(9 if nd_tag == "nd1" else 5), name=nd_tag)
            nc.scalar.activation(nd[:], nd_p[:, 0:N], AF.Copy)
            nds.append(nd)
        return nds

    class _TopkChunk:
        """One 128-node topk chunk.  .dve() emits the DVE top-16 ops; .chain()
        emits idx transpose -> idx copy -> quarter dma_gather.  The two parts
        are emitted at different points of the consumer W-stage so the PE
        transpose never head-of-line-blocks W matmuls (and vice versa)."""

        def __init__(self, nd, idxTp, idxs, gtab_ap, xjg, c):
            self.__dict__.update(nd=nd, idxTp=idxTp, idxs=idxs,
                                 gtab_ap=gtab_ap, xjg=xjg, c=c)

        def dve(self):
            nd = self.nd
            maxv = sb.tile([128, 16], F32, tag="maxv", name="maxv")
            maxi = sb.tile([128, 16], U16, tag="maxi", name="maxi")
            nc.vector.max(out=maxv[:, 0:8], in_=nd[:])
            nc.vector.max_index(out=maxi[:, 0:8], in_max=maxv[:, 0:8], in_values=nd[:])
            nc.vector.match_replace(out=nd[:], in_to_replace=maxv[:, 0:8],
                                    in_values=nd[:], imm_value=-1e30)
            nc.vector.max(out=maxv[:, 8:16], in_=nd[:])
            nc.vector.max_index(out=maxi[:, 8:16], in_max=maxv[:, 8:16], in_values=nd[:])
            mif = sb.tile([128, 16], F16, tag="mif", name="mif")
            nc.vector.tensor_copy(mif[:], maxi[:])
            self.mif = mif

        def chain(self):
            """Transpose this chunk's indices; on odd chunks launch a
            half-gather covering chunks c-1 and c (fewer cross-engine hops
            than per-chunk gathers — HW latency is hop-dominated)."""
            QI = K * 128
            c, idxs = self.c, self.idxs
            cs = slice(128 * c, 128 * (c + 1))
            nc.tensor.transpose(out=self.idxTp[:, cs], in_=self.mif[:],
                                identity=ident[:])
            if c % 2 == 1:
                hs = slice(128 * (c - 1), 128 * (c + 1))
                nc.scalar.activation(idxs[0:16, hs], self.idxTp[:, hs], AF.Copy)
                nc.sync.dma_start(out=idxs[16:32, hs], in_=idxs[0:16, hs])
                nc.gpsimd.dma_gather(
                    out_ap=self.xjg[:, None, QI * (c - 1):QI * (c + 1)],
                    in_ap=self.gtab_ap, idxs_ap=idxs[:, hs],
                    num_idxs=2 * QI, num_idxs_reg=2 * QI, elem_size=128,
                    transpose=True, single_packet=False)

    def make_topk_chunks(nds, gtab_ap, xjg_tag):
        xjg = sb.tile([128, K * N], F16, tag=xjg_tag, bufs=3, name=xjg_tag)
        idxTp = ps.tile([16, N], F16, tag="idxTp", name="idxTp")
        idxs = idx_tiles[idx_slot[0] % NIDX_SLOTS]
        idx_slot[0] += 1
        return xjg, [_TopkChunk(nds[c], idxTp, idxs, gtab_ap, xjg, c)
                     for c in range(NCHUNK)]

    # ----------------- A1: conv1 score prep -----------------
    # B1 layout: row 0 = -|f|^2/2 (Act-writable partition 0), rows 1:6 = feat.
    # Zero rows in w1modz / wl1xxz / ones6z null out row 0 where unwanted.
    def prefetch_B(g):
        B = sb.tile([6, N], F16, tag="B1", bufs=5, name="B1")
        nc.sync.dma_start(out=B[1:6, :], in_=t["feat5h"][g])
        nc.gpsimd.memset(B[0:1, :], 0.0)
        st.setdefault(g, {})["B1"] = B

    def A1a(g):
        if g + 1 < G:
            prefetch_B(g + 1)
        B = st[g]["B1"]
        sc = sb.tile([6, N], F16, tag="sc1", name="sc1")
        nc.scalar.activation(sc[:], B[:], AF.Copy)
        nc.scalar.activation(sc[0:1, :], onesrow[:], AF.Copy)
        F2 = sb.tile([6, N], F16, tag="F21", name="F21")
        nc.scalar.activation(F2[:], B[:], AF.Square)
        sqp = ps.tile([1, N], F32, tag="ms", name="sqp1")
        nc.tensor.matmul(out=sqp[:], lhsT=ones6z[:], rhs=F2[:],
                         start=True, stop=True)
        nc.scalar.activation(B[0:1, :], sqp[:], AF.Copy, scale=-0.5)
        st[g]["nds1"] = score_block(g, 5, sc, B, "nd1")

    # ----------------- W1: conv1 edge MLP -> S2 + x1tab -----------------
    def W1(g):
        B = st[g]["B1"]
        xjg = st[g].pop("xjg1")
        S2 = sb.tile([65, N], F16, tag="S2", bufs=3, name="S2")
        st[g]["S2"] = S2

        a1s, a2s = {}, {}

        def l1(s):
            a1 = ps.tile([128, 1024], F32, tag="a1", bufs=2, name="a1")
            a1s[s] = a1
            for h in range(2):
                for r in range(2):
                    c = 4 * s + 2 * h + r
                    out = a1[64 * h:64 * h + 64, 512 * r:512 * (r + 1)]
                    nc.tensor.matmul(out=out, lhsT=w["w1modz"][:],
                                     rhs=B[0:6, 32 * c:32 * (c + 1), None]
                                     .to_broadcast([6, 32, K]),
                                     start=True, stop=False)
                    nc.tensor.matmul(out=out, lhsT=w["w1botp"][:],
                                     rhs=xjg[:, 512 * c:512 * (c + 1)],
                                     start=False, stop=True)

        def l2(s):
            a1 = a1s.pop(s)
            h1 = sb.tile([128, 1024], F16, tag="h1", bufs=4, name="h1")
            nc.scalar.activation(h1[:], a1[:], AF.Prelu, bias=w["b1a2"][:],
                                 alpha=0.01)
            a2 = ps.tile([128, 1024], F32, tag="a2", name="a2")
            a2s[s] = a2
            for h in range(2):
                for r in range(2):
                    nc.tensor.matmul(
                        out=a2[64 * h:64 * h + 64, 512 * r:512 * (r + 1)],
                        lhsT=w["w1b2"][64 * h:64 * h + 64, :],
                        rhs=h1[64 * h:64 * h + 64, 512 * r:512 * (r + 1)],
                        start=True, stop=True)

        def kout(tt):
            a2 = a2s.pop(tt)
            kmx = sb.tile([128, 64], F32, tag="kmx", name="kmx")
            nc.vector.tensor_reduce(out=kmx[:], in_=a2[:].rearrange(
                "p (m k) -> p m k", k=K), op=ALU.max, axis=AX.X)
            for h in range(2):
                cols = slice(128 * tt + 64 * h, 128 * tt + 64 * h + 64)
                nc.scalar.activation(S2[0:64, cols], kmx[64 * h:64 * h + 64, :],
                                     AF.Prelu, bias=w["b1b"][:], alpha=0.01)
            # node-major staging for the conv2 gather table
            Tp = ps.tile([128, 64], F16, tag="ms", name="Tp")
            nc.tensor.transpose(out=Tp[:], in_=S2[0:64, 128 * tt:128 * (tt + 1)],
                                identity=ident[0:64, 0:64])
            xnm = sb.tile([128, 64], F16, tag="xnm", bufs=3, name="xnm")
            nc.scalar.activation(xnm[:], Tp[:], AF.Copy)
            nc.sync.dma_start(out=x1tab[g][128 * tt:128 * (tt + 1), 0:64], in_=xnm[:])
            nc.sync.dma_start(out=x1tab[g][128 * tt:128 * (tt + 1), 64:128], in_=xnm[:])

        # one-super lookahead so PE never waits on the Act Prelu
        l1(0)
        for s in range(1, 4):
            l1(s)
            l2(s - 1)
            kout(s - 1)
        l2(3)
        kout(3)

    # ----------------- A2: conv2 score / topk+gather -----------------
    def A2a(g):
        S2 = st[g]["S2"]
        sc2 = sb.tile([65, N], F16, tag="sc2", name="sc2")
        nc.scalar.activation(sc2[0:64, :], S2[0:64, :], AF.Copy)
        nc.scalar.activation(sc2[64:65, :], onesrow[:], AF.Copy)
        F22 = sb.tile([64, N], F16, tag="F22", name="F22")
        nc.scalar.activation(F22[:], S2[0:64, :], AF.Square)
        sqp = ps.tile([1, N], F32, tag="ms", name="sqp2")
        nc.tensor.matmul(out=sqp[:], lhsT=ones64[:], rhs=F22[:],
                         start=True, stop=True)
        nc.scalar.activation(S2[64:65, :], sqp[:], AF.Copy, scale=-0.5)
        st[g]["nds2"] = score_block(g, 64, sc2, S2, "nd2")

    # ----------------- W2: conv2 edge MLP -----------------
    def W2a(g):
        S2 = st[g]["S2"]
        xjg = st[g].pop("xjg2")
        x2 = sb.tile([64, N], F16, tag="x2", name="x2")
        st[g]["x2"] = x2

        a1s, a2s = {}, {}

        def l1(s):
            a1 = ps.tile([128, 1024], F32, tag="a1", bufs=2, name="a1c2")
            a1s[s] = a1
            for r in range(2):
                c = 2 * s + r
                out = a1[:, 512 * r:512 * (r + 1)]
                nc.tensor.matmul(out=out, lhsT=w["w2mod"][:],
                                 rhs=S2[0:64, 32 * c:32 * (c + 1), None]
                                 .to_broadcast([64, 32, K]),
                                 start=True, stop=False)
                nc.tensor.matmul(out=out, lhsT=w["w2botp"][:],
                                 rhs=xjg[:, 512 * c:512 * (c + 1)],
                                 start=False, stop=True)

        def l2(s):
            a1 = a1s.pop(s)
            h1 = sb.tile([128, 1024], F16, tag="h1", bufs=4, name="h1c2")
            nc.scalar.activation(h1[:], a1[:], AF.Prelu, bias=w["b2a"][:],
                                 alpha=0.01)
            if s % 2 == 0:
                a2s[s // 2] = ps.tile([128, 1024], F32, tag="a2", name="a2c2")
            a2 = a2s[s // 2]
            hh = s % 2
            for r in range(2):
                nc.tensor.matmul(
                    out=a2[64 * hh:64 * hh + 64, 512 * r:512 * (r + 1)],
                    lhsT=w["w2b"][:], rhs=h1[:, 512 * r:512 * (r + 1)],
                    start=True, stop=True)

        def kout(tt):
            a2 = a2s.pop(tt)
            kmx = sb.tile([128, 64], F32, tag="kmx", name="kmx2")
            nc.vector.tensor_reduce(out=kmx[:], in_=a2[:].rearrange(
                "p (m k) -> p m k", k=K), op=ALU.max, axis=AX.X)
            for h in range(2):
                cols = slice(128 * tt + 64 * h, 128 * tt + 64 * h + 64)
                nc.scalar.activation(x2[:, cols], kmx[64 * h:64 * h + 64, :],
                                     AF.Prelu, bias=w["b2b"][:], alpha=0.01)

        l1(0)
        for s in range(1, 8):
            l1(s)
            l2(s - 1)
            if (s - 1) % 2 == 1:
                kout((s - 1) // 2)
        l2(7)
        kout(3)

    # ----------------- lin1 + global max pool -----------------
    def LIN(g):
        B = st[g]["B1"]
        S2 = st[g]["S2"]
        x2 = st[g]["x2"]
        hsbs = []
        for c in range(NCHUNK):
            hp = ps.tile([128, N], F32, tag="a1", bufs=2, name="hp")
            nc.tensor.matmul(out=hp[:], lhsT=w["wl1xxz"][:, 128 * c:128 * (c + 1)],
                             rhs=B[0:6, :], start=True, stop=False)
            nc.tensor.matmul(out=hp[:], lhsT=w["wl1x1"][:, 128 * c:128 * (c + 1)],
                             rhs=S2[0:64, :], start=False, stop=False)
            nc.tensor.matmul(out=hp[:], lhsT=w["wl1x2"][:, 128 * c:128 * (c + 1)],
                             rhs=x2[:], start=False, stop=True)
            hsb = sb.tile([128, N], F16, tag="hsb", bufs=8, name="hsb")
            nc.scalar.activation(hsb[:], hp[:], AF.Prelu,
                                 bias=w["bl1c"][:, c:c + 1], alpha=0.01)
            hsbs.append(hsb)
        for fo in range(2):
            h2p = ps.tile([128, N], F32, tag="a2", name="h2p")
            for c in range(NCHUNK):
                nc.tensor.matmul(out=h2p[:],
                                 lhsT=w["wl2"][:, c, 128 * fo:128 * (fo + 1)],
                                 rhs=hsbs[c][:], start=(c == 0),
                                 stop=(c == NCHUNK - 1))
            gt = Gt_lo if fo == 0 else Gt_hi
            nc.vector.tensor_reduce(out=gt[:, g:g + 1], in_=h2p[:], op=ALU.max,
                                    axis=AX.X)
        del st[g]

    # ----------------- software-pipelined graph loop -----------------
    # A1a(g) runs a full tstep before graph g's topk chunks so the score-prep
    # chain (Pool copies -> sqp -> B0 -> score mms -> nd copies) is never on
    # the iteration-boundary critical path.
    tk1, tk2 = {}, {}
    prefetch_B(0)
    A1a(0)
    for tstep in range(G + 2):
        if tstep + 1 < G:
            A1a(tstep + 1)
        if tstep < G:
            xjg, cks = make_topk_chunks(st[tstep].pop("nds1"),
                                        t["xtab1"][tstep], "xjg1")
            st[tstep]["xjg1"] = xjg
            tk1[tstep] = cks
        if 0 <= tstep - 1 < G:
            W1(tstep - 1)
            A2a(tstep - 1)
            xjg, cks2 = make_topk_chunks(st[tstep - 1].pop("nds2"),
                                         x1tab[tstep - 1], "xjg2")
            st[tstep - 1]["xjg2"] = xjg
            tk2[tstep - 1] = cks2
        if 0 <= tstep - 2 < G:
            W2a(tstep - 2)
        for ck in tk1.pop(tstep, []):
            ck.dve()
            ck.chain()
        for ck in tk2.pop(tstep - 1, []):
            ck.dve()
            ck.chain()
        if 0 <= tstep - 2 < G:
            LIN(tstep - 2)

    # ----------------- head -----------------
    t1p = ps.tile([128, G], F32, tag="ms", name="t1p")
    for fo in range(2):
        gt = Gt_lo if fo == 0 else Gt_hi
        ga = sb.tile([128, G], F16, tag="ga", name="ga")
        nc.scalar.activation(ga[:], gt[:], AF.Prelu, bias=w["bl2c"][:, fo:fo + 1],
                             alpha=0.01)
        nc.tensor.matmul(out=t1p[:], lhsT=w["wm1"][:, fo, :],
                         rhs=ga[:], start=(fo == 0), stop=(fo == 1))
    t1 = sb.tile([128, G], F16, tag="t1", name="t1")
    nc.scalar.activation(t1[:], t1p[:], AF.Prelu, bias=w["bm1"][:], alpha=0.01)
    outp = ps.tile([3, G], F32, tag="ms", name="outp")
    nc.tensor.matmul(out=outp[:], lhsT=w["wm2"][:], rhs=t1[:], start=True, stop=True)
    outsb = sb.tile([3, G], F32, tag="outsb", name="outsb")
    nc.scalar.activation(outsb[:], outp[:], AF.Identity, bias=w["bm2"][:])
    nc.sync.dma_start(out=t["o"][:], in_=outsb[:])

    if rep_ctx is not None:
        rep_ctx.__exit__(None, None, None)

    for pool in (dram, ps, sb, sbw):
        pool.release()


# ======================= harness entry point =======================
_CACHE = {}


def _get_program(G):
    if "nc" not in _CACHE:
        import concourse.bacc as _bacc
        nc = _bacc.Bacc()
        build(nc, G)
        _CACHE["nc"] = nc
    return _CACHE["nc"]


def kernel(x, pos, tq, batch, w1a, b1a, w1b, b1b, w2a, b2a, w2b, b2b,
           wl1, bl1, wl2, bl2, wm1, bm1, wm2, bm2):
    """Full-input entry: shards graphs over 8 NeuronCores, returns [128, 3]."""
    from concourse.bass_utils import run_bass_kernel_spmd
    inputs = dict(x=np.asarray(x), pos=np.asarray(pos), tq=np.asarray(tq),
                  w1a=np.asarray(w1a), b1a=np.asarray(b1a),
                  w1b=np.asarray(w1b), b1b=np.asarray(b1b),
                  w2a=np.asarray(w2a), b2a=np.asarray(b2a),
                  w2b=np.asarray(w2b), b2b=np.asarray(b2b),
                  wl1=np.asarray(wl1), bl1=np.asarray(bl1),
                  wl2=np.asarray(wl2), bl2=np.asarray(bl2),
                  wm1=np.asarray(wm1), bm1=np.asarray(bm1),
                  wm2=np.asarray(wm2), bm2=np.asarray(bm2))
    NCORES = 8
    B_all = inputs["x"].shape[0] // N
    G = B_all // NCORES
    nc = _get_program(G)
    in_maps = [host_prep(inputs, G, c) for c in range(NCORES)]
    res = run_bass_kernel_spmd(nc, in_maps, core_ids=list(range(NCORES)))
    out = np.concatenate([res.results[c]["o"].T for c in range(NCORES)], axis=0)
    return out.astype(np.float32)


# revision 67
# speedup vs baseline: 1.2951x; 1.1287x over previous
"""DGCNN (nn_DGCNN_type1) Trainium2 Bass kernel — self-contained.

Strategy: data-parallel over the 128 graphs, 16 per NeuronCore across 8 cores.
Per graph the chain is score-matmul -> DVE top-16 -> dma_gather -> edge MLP
(PSUM-accumulated f16 matmuls) -> k-max -> next conv -> lin1 + pool.  The
graph loop is software-pipelined two graphs deep at emission order so the
11.7us gathers and the DVE topk chain of graph t overlap the PE/Act edge-MLP
work of graphs t-1 / t-2.  All score math is f16 (PSUM f32 accumulate);
PSUM-releasing DVE ops (k-max, pool reduce) are emitted ahead of the topk
chains so PSUM pool rotation never waits on a topk.
"""

import numpy as np
import concourse.bacc as bacc
import concourse.mybir as mybir
from concourse.tile import TileContext
from concourse.masks import make_identity

F32, F16, I16, U16 = (mybir.dt.float32, mybir.dt.float16, mybir.dt.int16,
                      mybir.dt.uint16)
AF = mybir.ActivationFunctionType
ALU = mybir.AluOpType
AX = mybir.AxisListType

N = 512
K = 16
NCHUNK = N // 128  # 4 row-chunks for the NxN score matrix


def host_prep(inputs, G, core):
    """Build the per-core in_map (numpy only: layout/dtype prep, no model math)."""
    f16 = np.float16
    x, pos, tq = inputs["x"], inputs["pos"], inputs["tq"]
    B_all = x.shape[0] // N
    xx = np.concatenate([tq, x, pos], axis=1).reshape(B_all, N, 5).astype(np.float32)
    sl = slice(core * G, (core + 1) * G)
    xxc = xx[sl]                                   # [G, 512, 5]
    feat5 = np.ascontiguousarray(xxc.transpose(0, 2, 1))         # [G, 5, 512]
    xtab1 = np.zeros((G, N, 128), f16)
    xtab1[:, :, 0:5] = xxc.astype(f16)

    w1a, w1b = inputs["w1a"], inputs["w1b"]
    w2a, w2b = inputs["w2a"], inputs["w2b"]
    w1botp = np.zeros((128, 64), f16)
    w1botp[0:5] = w1a[5:10].astype(f16)
    w2botp = np.zeros((128, 128), f16)
    w2botp[0:64] = w2a[64:128].astype(f16)
    wl1 = inputs["wl1"]

    return {
        "feat5h": feat5.astype(f16),
        "xtab1": xtab1,
        "w1modz": np.ascontiguousarray(
            np.vstack([np.zeros((1, 64), np.float32),
                       w1a[0:5] - w1a[5:10]]).astype(f16)),
        "w1botp": w1botp,
        "w1b": np.ascontiguousarray(w1b.astype(f16)),
        "b1a2": np.tile(inputs["b1a"], 2).reshape(128, 1).astype(np.float32),
        "w1b2": np.ascontiguousarray(np.vstack([w1b, w1b]).astype(f16)),
        "b1b": inputs["b1b"].reshape(64, 1).astype(np.float32),
        "w2mod": np.ascontiguousarray((w2a[0:64] - w2a[64:128]).astype(f16)),
        "w2botp": w2botp,
        "w2b": np.ascontiguousarray(w2b.astype(f16)),
        "b2a": inputs["b2a"].reshape(128, 1).astype(np.float32),
        "b2b": inputs["b2b"].reshape(64, 1).astype(np.float32),
        "wl1xxz": np.ascontiguousarray(
            np.vstack([np.zeros((1, 512), np.float32), wl1[0:5]]).astype(f16)),
        "wl1x1": np.ascontiguousarray(wl1[5:69].astype(f16)),
        "wl1x2": np.ascontiguousarray(wl1[69:133].astype(f16)),
        "bl1c": np.ascontiguousarray(inputs["bl1"].reshape(4, 128).T.astype(np.float32)),
        "wl2": np.ascontiguousarray(inputs["wl2"].astype(f16).reshape(4, 128, 256).transpose(1, 0, 2)),
        "bl2c": np.ascontiguousarray(inputs["bl2"].reshape(2, 128).T.astype(np.float32)),
        "wm1": np.ascontiguousarray(inputs["wm1"].astype(f16).reshape(2, 128, 128).transpose(1, 0, 2)),
        "bm1": inputs["bm1"].reshape(128, 1).astype(np.float32),
        "wm2": np.ascontiguousarray(inputs["wm2"].astype(f16)),
        "bm2": inputs["bm2"].reshape(3, 1).astype(np.float32),
    }


def declare_io(nc, G):
    t = {}
    def inp(name, shape, dt):
        t[name] = nc.dram_tensor(name, shape, dt, kind="ExternalInput")
    inp("feat5h", [G, 5, N], F16)
    inp("xtab1", [G, N, 128], F16)
    inp("w1modz", [6, 64], F16); inp("w1botp", [128, 64], F16)
    inp("w1b", [64, 64], F16); inp("w1b2", [128, 64], F16)
    inp("b1a2", [128, 1], F32); inp("b1b", [64, 1], F32)
    inp("w2mod", [64, 128], F16); inp("w2botp", [128, 128], F16)
    inp("w2b", [128, 64], F16); inp("b2a", [128, 1], F32); inp("b2b", [64, 1], F32)
    inp("wl1xxz", [6, N], F16); inp("wl1x1", [64, N], F16); inp("wl1x2", [64, N], F16)
    inp("bl1c", [128, 4], F32); inp("wl2", [128, 4, 256], F16); inp("bl2c", [128, 2], F32)
    inp("wm1", [128, 2, 128], F16); inp("bm1", [128, 1], F32)
    inp("wm2", [128, 3], F16); inp("bm2", [3, 1], F32)
    t["o"] = nc.dram_tensor("o", [3, G], F32, kind="ExternalOutput")
    return t


def build(nc, G, reps=1):
    t = declare_io(nc, G)
    with TileContext(nc) as tc:
        _build_body(nc, tc, t, G, reps)
    nc.compile()
    return t


def _build_body(nc, tc, t, G, reps=1):
    sbw = tc.alloc_tile_pool(name="sbw", bufs=1)          # persistent
    sb = tc.alloc_tile_pool(name="sb", bufs=2)            # rotating tiles
    ps = tc.alloc_tile_pool(name="ps", bufs=1, space="PSUM")
    dram = tc.alloc_tile_pool(name="dram", bufs=1, space="DRAM")

    # ---- persistent weight tiles ----
    w = {}
    for name in ["w1modz", "w1botp", "w1b", "w1b2", "w2mod", "w2botp", "w2b",
                 "wl1xxz", "wl1x1", "wl1x2", "wl2", "wm1", "wm2"]:
        w[name] = sbw.tile(list(t[name].shape), F16, tag=name, name='w_' + name)
        nc.sync.dma_start(out=w[name][:], in_=t[name][:])
    for name in ["b1a2", "b1b", "b2a", "b2b", "bl1c", "bl2c", "bm1", "bm2"]:
        w[name] = sbw.tile(list(t[name].shape), F32, tag=name, name='b_' + name)
        nc.sync.dma_start(out=w[name][:], in_=t[name][:])
    ident = sbw.tile([128, 128], F16, tag="ident")
    make_identity(nc, ident[:])
    ones64 = sbw.tile([64, 1], F16, tag="ones64")
    nc.gpsimd.memset(ones64[:], 1.0)
    ones6z = sbw.tile([6, 1], F16, tag="ones6z")
    nc.gpsimd.memset(ones6z[:], 1.0)
    nc.gpsimd.memset(ones6z[0:1, :], 0.0)
    onesrow = sbw.tile([1, N], F16, tag="onesrow")
    nc.gpsimd.memset(onesrow[:], 1.0)

    # persistent gather-index tiles (rows 32-127 must hold valid values)
    NIDX_SLOTS = 3
    idx_tiles = []
    for s in range(NIDX_SLOTS):
        it = sbw.tile([128, N], I16, tag=f"idxs{s}", name=f"idxs{s}")
        nc.gpsimd.memset(it[:], 0)
        idx_tiles.append(it)
    idx_slot = [0]

    x1tab = dram.tile([G, N, 128], F16, tag="x1tab")

    Gt_lo = sbw.tile([128, G], F32, tag="gtlo")
    Gt_hi = sbw.tile([128, G], F32, tag="gthi")

    rep_ctx = tc.For_i(0, reps, 1) if reps > 1 else None
    if rep_ctx is not None:
        rep_ctx.__enter__()

    st = {}  # per-graph in-flight tiles

    # ----------------- stage bodies -----------------
    def score_block(g, dp, lhs_sc, rhs_B, nd_tag):
        """Emit sq row + 4 chunk score matmuls; returns list of SBUF nd tiles.
        score[p, f] = feat_p . feat_f - |feat_f|^2 / 2  (ordering == -d2/2)."""
        nds = []
        for c in range(NCHUNK):
            nd_p = ps.tile([128, 1024], F32, tag="a1", bufs=2, name="ndp")
            nc.tensor.matmul(out=nd_p[:, 0:N],
                             lhsT=lhs_sc[0:dp + 1, 128 * c:128 * (c + 1)],
                             rhs=rhs_B[0:dp + 1, :], start=True, stop=True)
            nd = sb.tile([128, N], F32, tag=nd_tag,
                         bufs=(9 if nd_tag == "nd1" else 5), name=nd_tag)
            nc.scalar.activation(nd[:], nd_p[:, 0:N], AF.Copy)
            nds.append(nd)
        return nds

    class _TopkChunk:
        """One 128-node topk chunk.  .dve() emits the DVE top-16 ops; .chain()
        emits idx transpose -> idx copy -> quarter dma_gather.  The two parts
        are emitted at different points of the consumer W-stage so the PE
        transpose never head-of-line-blocks W matmuls (and vice versa)."""

        def __init__(self, nd, idxTp, idxs, gtab_ap, xjg, c):
            self.__dict__.update(nd=nd, idxTp=idxTp, idxs=idxs,
                                 gtab_ap=gtab_ap, xjg=xjg, c=c)

        def dve(self):
            nd = self.nd
            maxv = sb.tile([128, 16], F32, tag="maxv", name="maxv")
            maxi = sb.tile([128, 16], U16, tag="maxi", name="maxi")
            nc.vector.max(out=maxv[:, 0:8], in_=nd[:])
            nc.vector.max_index(out=maxi[:, 0:8], in_max=maxv[:, 0:8], in_values=nd[:])
            nc.vector.match_replace(out=nd[:], in_to_replace=maxv[:, 0:8],
                                    in_values=nd[:], imm_value=-1e30)
            nc.vector.max(out=maxv[:, 8:16], in_=nd[:])
            nc.vector.max_index(out=maxi[:, 8:16], in_max=maxv[:, 8:16], in_values=nd[:])
            mif = sb.tile([128, 16], F16, tag="mif", name="mif")
            nc.vector.tensor_copy(mif[:], maxi[:])
            self.mif = mif

        def chain(self):
            """Transpose this chunk's indices; on odd chunks launch a
            half-gather covering chunks c-1 and c (fewer cross-engine hops
            than per-chunk gathers — HW latency is hop-dominated)."""
            QI = K * 128
            c, idxs = self.c, self.idxs
            cs = slice(128 * c, 128 * (c + 1))
            nc.tensor.transpose(out=self.idxTp[:, cs], in_=self.mif[:],
                                identity=ident[:])
            if c % 2 == 1:
                hs = slice(128 * (c - 1), 128 * (c + 1))
                nc.scalar.activation(idxs[0:16, hs], self.idxTp[:, hs], AF.Copy)
                nc.sync.dma_start(out=idxs[16:32, hs], in_=idxs[0:16, hs])
                nc.gpsimd.dma_gather(
                    out_ap=self.xjg[:, None, QI * (c - 1):QI * (c + 1)],
                    in_ap=self.gtab_ap, idxs_ap=idxs[:, hs],
                    num_idxs=2 * QI, num_idxs_reg=2 * QI, elem_size=128,
                    transpose=True, single_packet=False)

    def make_topk_chunks(nds, gtab_ap, xjg_tag):
        xjg = sb.tile([128, K * N], F16, tag=xjg_tag, name=xjg_tag)
        idxTp = ps.tile([16, N], F16, tag="idxTp", name="idxTp")
        idxs = idx_tiles[idx_slot[0] % NIDX_SLOTS]
        idx_slot[0] += 1
        return xjg, [_TopkChunk(nds[c], idxTp, idxs, gtab_ap, xjg, c)
                     for c in range(NCHUNK)]

    # ----------------- A1: conv1 score prep -----------------
    # B1 layout: row 0 = -|f|^2/2 (Act-writable partition 0), rows 1:6 = feat.
    # Zero rows in w1modz / wl1xxz / ones6z null out row 0 where unwanted.
    def prefetch_B(g):
        B = sb.tile([6, N], F16, tag="B1", bufs=5, name="B1")
        nc.sync.dma_start(out=B[1:6, :], in_=t["feat5h"][g])
        nc.gpsimd.memset(B[0:1, :], 0.0)
        st.setdefault(g, {})["B1"] = B

    def A1a(g):
        if g + 1 < G:
            prefetch_B(g + 1)
        B = st[g]["B1"]
        sc = sb.tile([6, N], F16, tag="sc1", name="sc1")
        nc.scalar.activation(sc[:], B[:], AF.Copy)
        nc.scalar.activation(sc[0:1, :], onesrow[:], AF.Copy)
        F2 = sb.tile([6, N], F16, tag="F21", name="F21")
        nc.scalar.activation(F2[:], B[:], AF.Square)
        sqp = ps.tile([1, N], F32, tag="ms", name="sqp1")
        nc.tensor.matmul(out=sqp[:], lhsT=ones6z[:], rhs=F2[:],
                         start=True, stop=True)
        nc.scalar.activation(B[0:1, :], sqp[:], AF.Copy, scale=-0.5)
        st[g]["nds1"] = score_block(g, 5, sc, B, "nd1")

    # ----------------- W1: conv1 edge MLP -> S2 + x1tab -----------------
    def W1(g):
        B = st[g]["B1"]
        xjg = st[g].pop("xjg1")
        S2 = sb.tile([65, N], F16, tag="S2", bufs=3, name="S2")
        st[g]["S2"] = S2

        a1s, a2s = {}, {}

        def l1(s):
            a1 = ps.tile([128, 1024], F32, tag="a1", bufs=2, name="a1")
            a1s[s] = a1
            for h in range(2):
                for r in range(2):
                    c = 4 * s + 2 * h + r
                    out = a1[64 * h:64 * h + 64, 512 * r:512 * (r + 1)]
                    nc.tensor.matmul(out=out, lhsT=w["w1modz"][:],
                                     rhs=B[0:6, 32 * c:32 * (c + 1), None]
                                     .to_broadcast([6, 32, K]),
                                     start=True, stop=False)
                    nc.tensor.matmul(out=out, lhsT=w["w1botp"][:],
                                     rhs=xjg[:, 512 * c:512 * (c + 1)],
                                     start=False, stop=True)

        def l2(s):
            a1 = a1s.pop(s)
            h1 = sb.tile([128, 1024], F16, tag="h1", bufs=3, name="h1")
            nc.scalar.activation(h1[:], a1[:], AF.Prelu, bias=w["b1a2"][:],
                                 alpha=0.01)
            a2 = ps.tile([128, 1024], F32, tag="a2", name="a2")
            a2s[s] = a2
            for h in range(2):
                for r in range(2):
                    nc.tensor.matmul(
                        out=a2[64 * h:64 * h + 64, 512 * r:512 * (r + 1)],
                        lhsT=w["w1b2"][64 * h:64 * h + 64, :],
                        rhs=h1[64 * h:64 * h + 64, 512 * r:512 * (r + 1)],
                        start=True, stop=True)

        def kout(tt):
            a2 = a2s.pop(tt)
            kmx = sb.tile([128, 64], F32, tag="kmx", name="kmx")
            nc.vector.tensor_reduce(out=kmx[:], in_=a2[:].rearrange(
                "p (m k) -> p m k", k=K), op=ALU.max, axis=AX.X)
            for h in range(2):
                cols = slice(128 * tt + 64 * h, 128 * tt + 64 * h + 64)
                nc.scalar.activation(S2[0:64, cols], kmx[64 * h:64 * h + 64, :],
                                     AF.Prelu, bias=w["b1b"][:], alpha=0.01)
            # node-major staging for the conv2 gather table
            Tp = ps.tile([128, 64], F16, tag="ms", name="Tp")
            nc.tensor.transpose(out=Tp[:], in_=S2[0:64, 128 * tt:128 * (tt + 1)],
                                identity=ident[0:64, 0:64])
            xnm = sb.tile([128, 64], F16, tag="xnm", bufs=3, name="xnm")
            nc.scalar.activation(xnm[:], Tp[:], AF.Copy)
            nc.sync.dma_start(out=x1tab[g][128 * tt:128 * (tt + 1), 0:64], in_=xnm[:])
            nc.sync.dma_start(out=x1tab[g][128 * tt:128 * (tt + 1), 64:128], in_=xnm[:])

        # one-super lookahead so PE never waits on the Act Prelu
        l1(0)
        for s in range(1, 4):
            l1(s)
            l2(s - 1)
            kout(s - 1)
        l2(3)
        kout(3)

    # ----------------- A2: conv2 score / topk+gather -----------------
    def A2a(g):
        S2 = st[g]["S2"]
        sc2 = sb.tile([65, N], F16, tag="sc2", name="sc2")
        nc.scalar.activation(sc2[0:64, :], S2[0:64, :], AF.Copy)
        nc.scalar.activation(sc2[64:65, :], onesrow[:], AF.Copy)
        F22 = sb.tile([64, N], F16, tag="F22", name="F22")
        nc.scalar.activation(F22[:], S2[0:64, :], AF.Square)
        sqp = ps.tile([1, N], F32, tag="ms", name="sqp2")
        nc.tensor.matmul(out=sqp[:], lhsT=ones64[:], rhs=F22[:],
                         start=True, stop=True)
        nc.scalar.activation(S2[64:65, :], sqp[:], AF.Copy, scale=-0.5)
        st[g]["nds2"] = score_block(g, 64, sc2, S2, "nd2")

    # ----------------- W2: conv2 edge MLP -----------------
    def W2a(g):
        S2 = st[g]["S2"]
        xjg = st[g].pop("xjg2")
        x2 = sb.tile([64, N], F16, tag="x2", name="x2")
        st[g]["x2"] = x2

        a1s, a2s = {}, {}

        def l1(s):
            a1 = ps.tile([128, 1024], F32, tag="a1", bufs=2, name="a1c2")
            a1s[s] = a1
            for r in range(2):
                c = 2 * s + r
                out = a1[:, 512 * r:512 * (r + 1)]
                nc.tensor.matmul(out=out, lhsT=w["w2mod"][:],
                                 rhs=S2[0:64, 32 * c:32 * (c + 1), None]
                                 .to_broadcast([64, 32, K]),
                                 start=True, stop=False)
                nc.tensor.matmul(out=out, lhsT=w["w2botp"][:],
                                 rhs=xjg[:, 512 * c:512 * (c + 1)],
                                 start=False, stop=True)

        def l2(s):
            a1 = a1s.pop(s)
            h1 = sb.tile([128, 1024], F16, tag="h1", bufs=3, name="h1c2")
            nc.scalar.activation(h1[:], a1[:], AF.Prelu, bias=w["b2a"][:],
                                 alpha=0.01)
            if s % 2 == 0:
                a2s[s // 2] = ps.tile([128, 1024], F32, tag="a2", name="a2c2")
            a2 = a2s[s // 2]
            hh = s % 2
            for r in range(2):
                nc.tensor.matmul(
                    out=a2[64 * hh:64 * hh + 64, 512 * r:512 * (r + 1)],
                    lhsT=w["w2b"][:], rhs=h1[:, 512 * r:512 * (r + 1)],
                    start=True, stop=True)

        def kout(tt):
            a2 = a2s.pop(tt)
            kmx = sb.tile([128, 64], F32, tag="kmx", name="kmx2")
            nc.vector.tensor_reduce(out=kmx[:], in_=a2[:].rearrange(
                "p (m k) -> p m k", k=K), op=ALU.max, axis=AX.X)
            for h in range(2):
                cols = slice(128 * tt + 64 * h, 128 * tt + 64 * h + 64)
                nc.scalar.activation(x2[:, cols], kmx[64 * h:64 * h + 64, :],
                                     AF.Prelu, bias=w["b2b"][:], alpha=0.01)

        l1(0)
        for s in range(1, 8):
            l1(s)
            l2(s - 1)
            if (s - 1) % 2 == 1:
                kout((s - 1) // 2)
        l2(7)
        kout(3)

    # ----------------- lin1 + global max pool -----------------
    def LIN(g):
        B = st[g]["B1"]
        S2 = st[g]["S2"]
        x2 = st[g]["x2"]
        hsbs = []
        for c in range(NCHUNK):
            hp = ps.tile([128, N], F32, tag="a1", bufs=2, name="hp")
            nc.tensor.matmul(out=hp[:], lhsT=w["wl1xxz"][:, 128 * c:128 * (c + 1)],
                             rhs=B[0:6, :], start=True, stop=False)
            nc.tensor.matmul(out=hp[:], lhsT=w["wl1x1"][:, 128 * c:128 * (c + 1)],
                             rhs=S2[0:64, :], start=False, stop=False)
            nc.tensor.matmul(out=hp[:], lhsT=w["wl1x2"][:, 128 * c:128 * (c + 1)],
                             rhs=x2[:], start=False, stop=True)
            hsb = sb.tile([128, N], F16, tag="hsb", bufs=8, name="hsb")
            nc.scalar.activation(hsb[:], hp[:], AF.Prelu,
                                 bias=w["bl1c"][:, c:c + 1], alpha=0.01)
            hsbs.append(hsb)
        for fo in range(2):
            h2p = ps.tile([128, N], F32, tag="a2", name="h2p")
            for c in range(NCHUNK):
                nc.tensor.matmul(out=h2p[:],
                                 lhsT=w["wl2"][:, c, 128 * fo:128 * (fo + 1)],
                                 rhs=hsbs[c][:], start=(c == 0),
                                 stop=(c == NCHUNK - 1))
            gt = Gt_lo if fo == 0 else Gt_hi
            nc.vector.tensor_reduce(out=gt[:, g:g + 1], in_=h2p[:], op=ALU.max,
                                    axis=AX.X)
        del st[g]

    # ----------------- software-pipelined graph loop -----------------
    # A1a(g) runs a full tstep before graph g's topk chunks so the score-prep
    # chain (Pool copies -> sqp -> B0 -> score mms -> nd copies) is never on
    # the iteration-boundary critical path.
    tk1, tk2 = {}, {}
    prefetch_B(0)
    A1a(0)
    for tstep in range(G + 2):
        if tstep + 1 < G:
            A1a(tstep + 1)
        if tstep < G:
            xjg, cks = make_topk_chunks(st[tstep].pop("nds1"),
                                        t["xtab1"][tstep], "xjg1")
            st[tstep]["xjg1"] = xjg
            tk1[tstep] = cks
        if 0 <= tstep - 1 < G:
            W1(tstep - 1)
            A2a(tstep - 1)
            xjg, cks2 = make_topk_chunks(st[tstep - 1].pop("nds2"),
                                         x1tab[tstep - 1], "xjg2")
            st[tstep - 1]["xjg2"] = xjg
            tk2[tstep - 1] = cks2
        if 0 <= tstep - 2 < G:
            W2a(tstep - 2)
        for ck in tk1.pop(tstep, []):
            ck.dve()
            ck.chain()
        for ck in tk2.pop(tstep - 1, []):
            ck.dve()
            ck.chain()
        if 0 <= tstep - 2 < G:
            LIN(tstep - 2)

    # ----------------- head -----------------
    t1p = ps.tile([128, G], F32, tag="ms", name="t1p")
    for fo in range(2):
        gt = Gt_lo if fo == 0 else Gt_hi
        ga = sb.tile([128, G], F16, tag="ga", name="ga")
        nc.scalar.activation(ga[:], gt[:], AF.Prelu, bias=w["bl2c"][:, fo:fo + 1],
                             alpha=0.01)
        nc.tensor.matmul(out=t1p[:], lhsT=w["wm1"][:, fo, :],
                         rhs=ga[:], start=(fo == 0), stop=(fo == 1))
    t1 = sb.tile([128, G], F16, tag="t1", name="t1")
    nc.scalar.activation(t1[:], t1p[:], AF.Prelu, bias=w["bm1"][:], alpha=0.01)
    outp = ps.tile([3, G], F32, tag="ms", name="outp")
    nc.tensor.matmul(out=outp[:], lhsT=w["wm2"][:], rhs=t1[:], start=True, stop=True)
    outsb = sb.tile([3, G], F32, tag="outsb", name="outsb")
    nc.scalar.activation(outsb[:], outp[:], AF.Identity, bias=w["bm2"][:])
    nc.sync.dma_start(out=t["o"][:], in_=outsb[:])

    if rep_ctx is not None:
        rep_ctx.__exit__(None, None, None)

    for pool in (dram, ps, sb, sbw):
        pool.release()


# ======================= harness entry point =======================
_CACHE = {}


def _get_program(G):
    if "nc" not in _CACHE:
        import concourse.bacc as _bacc
        nc = _bacc.Bacc()
        build(nc, G)
        _CACHE["nc"] = nc
    return _CACHE["nc"]


def kernel(x, pos, tq, batch, w1a, b1a, w1b, b1b, w2a, b2a, w2b, b2b,
           wl1, bl1, wl2, bl2, wm1, bm1, wm2, bm2):
    """Full-input entry: shards graphs over 8 NeuronCores, returns [128, 3]."""
    from concourse.bass_utils import run_bass_kernel_spmd
    inputs = dict(x=np.asarray(x), pos=np.asarray(pos), tq=np.asarray(tq),
                  w1a=np.asarray(w1a), b1a=np.asarray(b1a),
                  w1b=np.asarray(w1b), b1b=np.asarray(b1b),
                  w2a=np.asarray(w2a), b2a=np.asarray(b2a),
                  w2b=np.asarray(w2b), b2b=np.asarray(b2b),
                  wl1=np.asarray(wl1), bl1=np.asarray(bl1),
                  wl2=np.asarray(wl2), bl2=np.asarray(bl2),
                  wm1=np.asarray(wm1), bm1=np.asarray(bm1),
                  wm2=np.asarray(wm2), bm2=np.asarray(bm2))
    NCORES = 8
    B_all = inputs["x"].shape[0] // N
    G = B_all // NCORES
    nc = _get_program(G)
    in_maps = [host_prep(inputs, G, c) for c in range(NCORES)]
    res = run_bass_kernel_spmd(nc, in_maps, core_ids=list(range(NCORES)))
    out = np.concatenate([res.results[c]["o"].T for c in range(NCORES)], axis=0)
    return out.astype(np.float32)
